# revision 1
# baseline (speedup 1.0000x reference)
"""Multi-head attention (QKV proj + rotary + softmax attention + out proj)
for Trainium2, sharded over 8 NeuronCores.

Problem: x[2,2048,1024], 16 heads x dh=64, rotary embedding, softmax
attention, output projection + bias.

Sharding: batch x head-group. Core c handles batch c//4 and the 4 heads
[4*(c%4), 4*(c%4)+4). Each core computes its QKV slice, rotary, attention,
and a partial output projection; the host sums the 4 partial projections
per batch and adds the bias.

Device-side design (per core, everything in "transposed" layout):
  - qkvT = W @ x^T computed as f32r matmuls (full PE rate, tf32-ish
    precision): qT/kT produced as [dh-pair(128), n] tiles, v as natural
    [n, e] tiles.
  - rotary applied on the fp32 psum output via DVE: q*cos +
    pairswap(q*sin_pre), with the dh dimension stored interleaved
    ([0,32,1,33,...]) so rotate_half becomes an adjacent-lane
    stream_shuffle. Output f32r.
  - dots: scoresT[j,n] = krotT^T-slice @ qrotT, two heads packed in the
    128x128 PE array via tile_position row-tiling (K=64 each). fp32 psum.
  - softmax without max-subtraction (logits are O(+-6)): ACT exp over
    2-j-tile psum batches (N=1024 per ACTIVATE), output fp16.
  - AV: lhsT = [v | ones] (M=65, fp16) so row 64 accumulates the softmax
    denominators for free; fp32 psum accumulation over the 16 j-tiles.
  - normalize: reciprocal_approx_fast of the sums row, partition-broadcast
    via a DRAM round-trip DMA (K=1 ones-matmul on the tail block), one DVE
    multiply -> aoT (f32r).
  - output proj: y[n,d] accumulated over the two head-pair e-chunks, f32r;
    the last block's pair-0 partial goes to a separate output (y3a) summed
    on the host, so the kernel tail only runs the pair-1 projection.
"""
import sys

sys.path.insert(0, "/opt/trn_rl_repo")

import numpy as np

import concourse.bacc as bacc
import concourse.tile as tile
from concourse import mybir
from concourse.bass_utils import run_bass_kernel_spmd

F32 = mybir.dt.float32
F32R = mybir.dt.float32r
BF16 = mybir.dt.bfloat16
FP16 = mybir.dt.float16
EXP = mybir.ActivationFunctionType.Exp
MULT = mybir.AluOpType.mult
ADD = mybir.AluOpType.add

B, N, DIM = 2, 2048, 1024
H, DH = 16, 64
INNER = H * DH
SCALE = DH ** -0.5
NCORES = 8
HPC = H // (NCORES // B)      # heads per core = 4
NPAIR = HPC // 2              # head pairs per core = 2

P = 128
NT = N // 512                 # 4 n-tiles of 512
DC = DIM // P                 # 8 d-chunks
JTILES = N // P               # 16 j-tiles
JB = JTILES // 2              # 8 j-batches (2 j-tiles each)

PAIRSWAP = [i ^ 1 for i in range(32)]

_CACHE = {}


def _build():
    nc = bacc.Bacc(None, target_bir_lowering=False, debug=False)
    with tile.TileContext(nc) as tc:
        with tc.tile_pool(name="dram", bufs=1, space="DRAM") as dram, \
             tc.tile_pool(name="const", bufs=1) as const, \
             tc.tile_pool(name="perst", bufs=1) as perst, \
             tc.tile_pool(name="tmp", bufs=1) as tmp, \
             tc.tile_pool(name="ps", bufs=1, space="PSUM") as ps:
            # ---------------- DRAM I/O ----------------
            xT_d = dram.tile([DIM, N], F32R, kind="ExternalInput", name="xT", uniquify=False)
            wqkT_d = dram.tile([DIM, 512], F32R, kind="ExternalInput", name="wqkT", uniquify=False)
            wvT_d = dram.tile([DIM, 256], F32R, kind="ExternalInput", name="wvT", uniquify=False)
            cq_d = dram.tile([P, N], F32, kind="ExternalInput", name="cq", uniquify=False)
            sq_d = dram.tile([P, N], F32, kind="ExternalInput", name="sq", uniquify=False)
            ck_d = dram.tile([P, N], F32, kind="ExternalInput", name="ck", uniquify=False)
            sk_d = dram.tile([P, N], F32, kind="ExternalInput", name="sk", uniquify=False)
            woT_d = dram.tile([256, DIM], F32R, kind="ExternalInput", name="woT", uniquify=False)
            y_d = dram.tile([N, DIM], F32, kind="ExternalOutput", name="y", uniquify=False)
            y3a_d = dram.tile([512, DIM], F32, kind="ExternalOutput", name="y3a", uniquify=False)

            # ---------------- constants to SBUF ----------------
            wqk_r = wqkT_d.rearrange("(c p) e -> p c e", p=P)
            wqk_sb = []
            for ech in (2, 0, 3, 1):    # k0, q0, k1, q1 arrival order
                w = const.tile([P, DC, P], F32R, name=f"wqk{ech}")
                nc.sync.dma_start(w[:, :, :], wqk_r[:, :, ech * P:(ech + 1) * P])
                wqk_sb.append((ech, w))
            wqk_sb = [w for _, w in sorted(wqk_sb)]
            wv_sb = const.tile([P, DC, 256], F32R)
            nc.sync.dma_start(wv_sb[:, :, :], wvT_d.rearrange("(c p) e -> p c e", p=P))
            wo_sb = const.tile([P, NPAIR, DIM], F32R)
            nc.sync.dma_start(wo_sb[:, :, :], woT_d.rearrange("(c p) d -> p c d", p=P))
            cq_sb = const.tile([P, N], F32)
            nc.sync.dma_start(cq_sb[:, :], cq_d[:, :])
            sq_sb = const.tile([P, N], F32)
            nc.sync.dma_start(sq_sb[:, :], sq_d[:, :])
            ck_sb = const.tile([P, N], F32)
            nc.sync.dma_start(ck_sb[:, :], ck_d[:, :])
            sk_sb = const.tile([P, N], F32)
            nc.sync.dma_start(sk_sb[:, :], sk_d[:, :])

            ones_f = const.tile([1, 64], F32)
            nc.vector.memset(ones_f[:, :], 1.0)
            ones_r = const.tile([1, 64], F32R)
            nc.vector.tensor_copy(ones_r[:, :], ones_f[:, :])

            # ---------------- persistent tiles ----------------
            qrot = [[perst.tile([P, 512], F32R, name=f"qrot{p}_{t}")
                     for t in range(NT)] for p in range(NPAIR)]
            krot = [[perst.tile([P, 512], F32R, name=f"krot{p}_{t}")
                     for t in range(NT)] for p in range(NPAIR)]
            v_aug = [perst.tile([P, 4, HPC, 65], FP16, name=f"vaug{t}")
                     for t in range(NT)]
            for t in range(NT):
                nc.vector.memset(v_aug[t][:, :, :, 64:65], 1.0)
            aoT = [[perst.tile([P, 512], F32R, name=f"aoT{p}_{t}")
                    for t in range(NT)] for p in range(NPAIR)]

            # ---------------- helpers ----------------
            xT_r = xT_d.rearrange("(c p) n -> p c n", p=P)

            def load_x(t):
                # per-d-chunk tiles so matmuls can start as soon as the first
                # 256KB chunk lands instead of waiting for the full 2MB tile
                xt = [tmp.tile([P, 512], F32R, name=f"xt{c}", tag=f"xt{c}", bufs=2)
                      for c in range(DC)]
                for c in range(DC):
                    nc.sync.dma_start(xt[c][:, :], xT_r[:, c, t * 512:(t + 1) * 512])
                return xt

            def qk_chunk(ech, t, xt, dest, cos_sb, sin_sb):
                # qkvT e-chunk [128, 512] = W-chunk @ xT-tile, then rotary.
                pqk = ps.tile([P, 512], F32, name="pqk", tag="m", bufs=2)
                for c in range(DC):
                    nc.tensor.matmul(pqk[:, :],
                                     wqk_sb[ech][:, c, :],
                                     xt[c][:, :],
                                     start=(c == 0), stop=(c == DC - 1))
                sl = slice(t * 512, (t + 1) * 512)
                t1 = tmp.tile([P, 512], F32, name="t1", tag="t1", bufs=2)
                t2 = tmp.tile([P, 512], F32, name="t2", tag="t2", bufs=2)
                t3 = tmp.tile([P, 512], F32, name="t3", tag="t3", bufs=2)
                nc.vector.tensor_tensor(t1[:, :], pqk[:, :], cos_sb[:, sl], op=MULT)
                nc.vector.tensor_tensor(t2[:, :], pqk[:, :], sin_sb[:, sl], op=MULT)
                nc.vector.stream_shuffle(t3[:, :], t2[:, :], PAIRSWAP)
                nc.vector.tensor_tensor(dest[:, :], t1[:, :], t3[:, :], op=ADD)

            def v_tile(t, xt):
                # v natural [n, e] for the 4 local heads, by 128-row subtiles
                for nsub in range(4):
                    pv = ps.tile([P, 256], F32, name="pv", tag="m", bufs=2)
                    for c in range(DC):
                        nc.tensor.matmul(pv[:, :],
                                         xt[c][:, nsub * P:(nsub + 1) * P],
                                         wv_sb[:, c, :],
                                         start=(c == 0), stop=(c == DC - 1))
                    nc.vector.tensor_copy(
                        v_aug[t][:, nsub, :, 0:64],
                        pv[:, :].rearrange("p (h d) -> p h d", h=HPC))

            def qkv_for_tile(t, ops):
                xt = load_x(t)
                for op in ops:
                    if op == "k0":
                        qk_chunk(2, t, xt, krot[0][t], ck_sb, sk_sb)
                    elif op == "k1":
                        qk_chunk(3, t, xt, krot[1][t], ck_sb, sk_sb)
                    elif op == "q0":
                        qk_chunk(0, t, xt, qrot[0][t], cq_sb, sq_sb)
                    elif op == "q1":
                        qk_chunk(1, t, xt, qrot[1][t], cq_sb, sq_sb)
                    elif op == "v":
                        v_tile(t, xt)

            def attention(nq, pair, pre_jb=None, mid_jb=None):
                pav = [ps.tile([65, 512], F32, name=f"pav{h}", tag="av", bufs=2)
                       for h in range(2)]
                for jb in range(JB):
                    if pre_jb is not None:
                        pre_jb(jb)
                    sc = [ps.tile([P, 2, 512], F32, name=f"sc{h}", tag="s", bufs=2)
                          for h in range(2)]
                    for jl in range(2):
                        jt = jb * 2 + jl
                        kt = krot[pair][jt // 4]
                        jsl = slice((jt % 4) * P, (jt % 4 + 1) * P)
                        for h in range(2):
                            rows = slice(h * 64, (h + 1) * 64)
                            nc.tensor.matmul(sc[h][:, jl, :],
                                             kt[rows, jsl],
                                             qrot[pair][nq][rows, :],
                                             start=True, stop=True,
                                             tile_position=(h * 64, 0))
                    ex = [tmp.tile([P, 2, 512], FP16, name=f"ex{h}", tag="ex", bufs=4)
                          for h in range(2)]
                    for h in range(2):
                        nc.scalar.activation(ex[h][:, :, :], sc[h][:, :, :], EXP)
                    if mid_jb is not None:
                        mid_jb(jb)
                    for jl in range(2):
                        jt = jb * 2 + jl
                        for h in range(2):
                            nc.tensor.matmul(pav[h][:, :],
                                             v_aug[jt // 4][:, jt % 4, pair * 2 + h, :],
                                             ex[h][:, jl, :],
                                             start=(jt == 0), stop=(jt == JTILES - 1))
                for h in range(2):
                    # evacuate psum immediately so the next (nq, pair) can start;
                    # sums row copied separately so it lands at partition 0
                    # (custom-DVE reciprocal_approx_fast requires base_partition 0)
                    av_sb = tmp.tile([64, 512], F32, name="av_sb", tag="avs", bufs=3)
                    sm_sb = tmp.tile([1, 512], F32, name="sm_sb", tag="sms", bufs=4)
                    nc.vector.tensor_copy(av_sb[:, :], pav[h][0:64, :])
                    nc.vector.tensor_copy(sm_sb[:, :], pav[h][64:65, :])
                    rc = tmp.tile([1, 512], F32, name="rc", tag="rc", bufs=2)
                    nc.vector.reciprocal_approx_fast(rc[:, :], sm_sb[:, :])
                    bc = tmp.tile([64, 512], F32, name="bc", tag="bc", bufs=2)
                    if nq == NT - 1:
                        # tail-critical: broadcast via K=1 ones-matmul (no DMA
                        # round-trip latency before the last y projection)
                        rcr = tmp.tile([1, 512], F32R, name="rcr", tag="rcr", bufs=2)
                        nc.vector.tensor_copy(rcr[:, :], rc[:, :])
                        pbc = ps.tile([64, 512], F32, name="pbc", tag="m", bufs=2)
                        nc.tensor.matmul(pbc[:, :], ones_r[:, :], rcr[:, :],
                                         start=True, stop=True)
                        nc.vector.tensor_copy(bc[:, :], pbc[:, :])
                    else:
                        # broadcast across partitions via a DRAM round-trip
                        rd = dram.tile([1, 512], F32, name="rd", tag="rd", bufs=2)
                        nc.sync.dma_start(rd[:, :], rc[:, :])
                        nc.sync.dma_start(bc[:, :], rd.to_broadcast([64, 512]))
                    rows = slice(h * 64, (h + 1) * 64)
                    nc.vector.tensor_tensor(aoT[pair][nq][rows, :],
                                            av_sb[:, :], bc[:, :], op=MULT)

            def y_proj_pair(nq, pair, out_d, row0):
                # single-pair partial projection (no cross-pair accumulation)
                for nsub in range(4):
                    ys = tmp.tile([P, DIM], F32, name="ysp", tag="ys", bufs=2)
                    nsl = slice(nsub * P, (nsub + 1) * P)
                    for dh2 in range(2):
                        py = ps.tile([P, 512], F32, name="pyp", tag="m", bufs=2)
                        dsl = slice(dh2 * 512, (dh2 + 1) * 512)
                        nc.tensor.matmul(py[:, :], aoT[pair][nq][:, nsl],
                                         wo_sb[:, pair, dsl],
                                         start=True, stop=True)
                        nc.vector.tensor_copy(ys[:, dsl], py[:, :])
                    nc.sync.dma_start(out_d[row0 + nsub * P:row0 + (nsub + 1) * P, :],
                                      ys[:, :])

            def y_proj(nq):
                for nsub in range(4):
                    ys = tmp.tile([P, DIM], F32, name="ys", tag="ys", bufs=2)
                    nsl = slice(nsub * P, (nsub + 1) * P)
                    for dh2 in range(2):
                        py = ps.tile([P, 512], F32, name="py", tag="m", bufs=2)
                        dsl = slice(dh2 * 512, (dh2 + 1) * 512)
                        for pair in range(NPAIR):
                            nc.tensor.matmul(py[:, :],
                                             aoT[pair][nq][:, nsl],
                                             wo_sb[:, pair, dsl],
                                             start=(pair == 0), stop=(pair == NPAIR - 1))
                        nc.vector.tensor_copy(ys[:, dsl], py[:, :])
                    nc.sync.dma_start(y_d[nq * 512 + nsub * P:
                                          nq * 512 + (nsub + 1) * P, :], ys[:, :])

            # ---------------- emission order ----------------
            # Tile has sequential program-order semantics: every tile must be
            # written (in emission order) before anything that reads it, and
            # per-psum-tag slot reuse is FIFO in emission order. QKV work and
            # the previous block's output projection are threaded just-in-time
            # through the attention j-loops: k before the dots that need it,
            # v between exp and the AV that needs it, next-q early (ahead of
            # y in the shared psum-tag FIFO) so rotary completes before the
            # block boundary.
            qkv_for_tile(0, ["k0", "q0"])

            def pre_first(jb):
                if jb == 1:
                    qkv_for_tile(0, ["k1", "q1"])
                elif jb in (2, 4, 6):
                    qkv_for_tile(jb // 2, ["k0", "k1"])

            def mid_first(jb):
                if jb in (0, 2, 4, 6):
                    qkv_for_tile(jb // 2, ["v"])

            def make_pre_q(t):
                def pre(jb):
                    if jb == 1:
                        qkv_for_tile(t, ["q0", "q1"])
                return pre

            def make_pre(nq):
                def pre(jb):
                    if jb == 1 and nq + 1 < NT:
                        qkv_for_tile(nq + 1, ["q0", "q1"])
                    if jb == 4 and nq >= 1:
                        y_proj(nq - 1)
                return pre

            for nq in range(NT):
                for pair in range(NPAIR):
                    if nq == 0 and pair == 0:
                        attention(nq, pair, pre_jb=pre_first, mid_jb=mid_first)
                    elif nq == 0 and pair == 1:
                        attention(nq, pair, pre_jb=make_pre_q(1))
                    elif pair == 0:
                        attention(nq, pair, pre_jb=make_pre(nq))
                    elif nq == NT - 1:
                        def pre_y3a(jb):
                            if jb == 2:
                                y_proj_pair(NT - 1, 0, y3a_d, 0)
                        attention(nq, pair, pre_jb=pre_y3a)
                    else:
                        attention(nq, pair)
            y_proj_pair(NT - 1, 1, y_d, (NT - 1) * 512)
    nc.compile()
    return nc


def _host_prep(x, rotary_emb, w_qkv, w_out):
    """Build the 8 per-core input maps."""
    x = np.asarray(x, dtype=np.float32)
    rotary_emb = np.asarray(rotary_emb, dtype=np.float32)
    w_qkv = np.asarray(w_qkv, dtype=np.float32)
    w_out = np.asarray(w_out, dtype=np.float32)

    # interleaved dh permutation: new row 2i <- dim i, 2i+1 <- dim 32+i
    perm = np.empty(DH, dtype=np.int64)
    perm[0::2] = np.arange(32)
    perm[1::2] = np.arange(32) + 32
    pair_swap = np.arange(DH) ^ 1

    cos = np.cos(rotary_emb).T[perm]                      # [dh, n] permuted
    sin = np.sin(rotary_emb).T[perm]
    sign = np.where(perm < 32, -1.0, 1.0)[:, None].astype(np.float32)
    sin_eff = sign * sin
    sin_pre = sin_eff[pair_swap]                          # pre-swapped
    c2 = np.concatenate([cos, cos], axis=0)               # [128, n]
    s2 = np.concatenate([sin_pre, sin_pre], axis=0)
    cq = np.ascontiguousarray(SCALE * c2)
    sq = np.ascontiguousarray(SCALE * s2)
    ck = np.ascontiguousarray(c2)
    sk = np.ascontiguousarray(s2)

    in_maps = []
    for core in range(NCORES):
        b = core // (NCORES // B)
        g = core % (NCORES // B)
        heads = range(4 * g, 4 * g + HPC)
        q_rows = np.concatenate([h * DH + perm for h in heads])
        k_rows = np.concatenate([INNER + h * DH + perm for h in heads])
        v_rows = np.arange(2 * INNER + 4 * g * DH, 2 * INNER + (4 * g + HPC) * DH)
        wqkT = np.ascontiguousarray(w_qkv[np.concatenate([q_rows, k_rows])].T)
        wvT = np.ascontiguousarray(w_qkv[v_rows].T)
        woT = np.ascontiguousarray(w_out[:, 4 * g * DH:(4 * g + HPC) * DH].T)
        xT = np.ascontiguousarray(x[b].T)
        in_maps.append({
            "xT": xT, "wqkT": wqkT, "wvT": wvT,
            "cq": cq, "sq": sq, "ck": ck, "sk": sk, "woT": woT,
        })
    return in_maps


def kernel(x, rotary_emb, w_qkv, w_out, b_out, _trace=False):
    if "nc" not in _CACHE:
        _CACHE["nc"] = _build()
    nc = _CACHE["nc"]
    in_maps = _host_prep(x, rotary_emb, w_qkv, w_out)
    res = run_bass_kernel_spmd(nc, in_maps, core_ids=list(range(NCORES)),
                               trace=_trace)
    _CACHE["last_result"] = res
    y = np.zeros((B, N, DIM), dtype=np.float32)
    for core in range(NCORES):
        b = core // (NCORES // B)
        y[b] += res.results[core]["y"]
        y[b, (NT - 1) * 512:] += res.results[core]["y3a"]
    y += np.asarray(b_out, dtype=np.float32)[None, None, :]
    return y



# revision 12
# speedup vs baseline: 1.5448x; 1.5448x over previous
"""Multi-head attention (QKV proj + rotary + softmax attention + out proj)
for Trainium2, sharded over 8 NeuronCores.

Problem: x[2,2048,1024], 16 heads x dh=64, rotary embedding, softmax
attention, output projection + bias.

Sharding: batch x head-group. Core c handles batch c//4 and the 4 heads
[4*(c%4), 4*(c%4)+4). Each core computes its QKV slice, rotary, attention,
and a partial output projection; the host sums the 4 partial projections
per batch and adds the bias.

Device-side design (per core, everything in "transposed" layout):
  - all matmul operands are fp16 (PE runs fp16 at 1 col/cycle vs the
    2 cyc/col fp32_mode=HIGH path that f32r lowers to); accumulation
    stays fp32 in PSUM so precision is set by the 10-bit fp16 mantissa
    of the operands only.
  - qkvT = W @ x^T: qT/kT produced as [dh-pair(128), n] tiles, v as
    natural [n, e] tiles.
  - rotary applied on the fp32 psum output via DVE: q*cos +
    pairswap(q*sin_pre), with the dh dimension stored interleaved
    ([0,32,1,33,...]) so rotate_half becomes an adjacent-lane
    stream_shuffle. Output fp16.
  - dots: scoresT[j,n] = krotT^T-slice @ qrotT, two heads packed in the
    128x128 PE array via tile_position row-tiling (K=64 each). fp32 psum.
  - softmax without max-subtraction (logits are O(+-6)): ACT exp over
    2-j-tile psum batches (N=1024 per ACTIVATE), output fp16.
  - AV: lhsT = [v | ones] (M=65, fp16) so row 64 accumulates the softmax
    denominators for free; fp32 psum accumulation over the 16 j-tiles.
  - normalize: reciprocal_approx_fast of the sums row, partition-broadcast
    via a DRAM round-trip DMA (K=1 ones-matmul on the tail block), one DVE
    multiply -> aoT (fp16).
  - output proj: y[n,d] accumulated over the two head-pair e-chunks, fp16
    out; the last block's pair-0 partial goes to a separate output (y3a)
    summed on the host, so the kernel tail only runs the pair-1 projection.
  - DMA: weights/x prepped host-side into the exact SBUF layouts so every
    load is contiguous; loads are split across the two HWDGE rings (sync:
    wqk + x + recip round-trips, scalar: rotary tables + wv + wo + y
    stores) so x tiles don't queue behind the constant pool.
"""
import sys

sys.path.insert(0, "/opt/trn_rl_repo")

import numpy as np

import concourse.bacc as bacc
import concourse.tile as tile
from concourse import mybir
from concourse.bass_utils import run_bass_kernel_spmd

F32 = mybir.dt.float32
FP16 = mybir.dt.float16
EXP = mybir.ActivationFunctionType.Exp
MULT = mybir.AluOpType.mult
ADD = mybir.AluOpType.add

B, N, DIM = 2, 2048, 1024
H, DH = 16, 64
INNER = H * DH
SCALE = DH ** -0.5
NCORES = 8
HPC = H // (NCORES // B)      # heads per core = 4
NPAIR = HPC // 2              # head pairs per core = 2

P = 128
NT = N // 512                 # 4 n-tiles of 512
DC = DIM // P                 # 8 d-chunks
JTILES = N // P               # 16 j-tiles
JB = JTILES // 2              # 8 j-batches (2 j-tiles each)

PAIRSWAP = [i ^ 1 for i in range(32)]

_CACHE = {}


def _build():
    nc = bacc.Bacc(None, target_bir_lowering=False, debug=False)
    with tile.TileContext(nc) as tc:
        with tc.tile_pool(name="dram", bufs=1, space="DRAM") as dram, \
             tc.tile_pool(name="const", bufs=1) as const, \
             tc.tile_pool(name="perst", bufs=1) as perst, \
             tc.tile_pool(name="tmp", bufs=1) as tmp, \
             tc.tile_pool(name="ps", bufs=1, space="PSUM") as ps:
            # ---------------- DRAM I/O (all host-prearranged, contiguous) ---
            xT_d = dram.tile([P, NT * DC * 512], FP16, kind="ExternalInput", name="xh", uniquify=False)
            wqk_d = dram.tile([P, 4 * DC * P], FP16, kind="ExternalInput", name="wqkh", uniquify=False)
            wv_d = dram.tile([P, DC * 256], FP16, kind="ExternalInput", name="wvh", uniquify=False)
            wo_d = dram.tile([P, NPAIR * DIM], FP16, kind="ExternalInput", name="woh", uniquify=False)
            csq_d = dram.tile([P, 2 * N], FP16, kind="ExternalInput", name="csq", uniquify=False)
            csk_d = dram.tile([P, 2 * N], FP16, kind="ExternalInput", name="csk", uniquify=False)
            y_d = dram.tile([N, DIM], FP16, kind="ExternalOutput", name="y", uniquify=False)
            y3a_d = dram.tile([512, DIM], FP16, kind="ExternalOutput", name="y3a", uniquify=False)

            # ---------------- constants to SBUF ----------------
            # sync ring: wqk (k0 first, matching first-use order); scalar
            # ring: rotary tables + wv + wo, also in first-use order.
            wqk_r = wqk_d.rearrange("p (a c e) -> p a c e", a=4, c=DC)
            wqk_sb = []
            for ech in (2, 0, 3, 1):    # k0, q0, k1, q1 arrival order
                w = const.tile([P, DC, P], FP16, name=f"wqk{ech}")
                nc.sync.dma_start(w[:, :, :], wqk_r[:, ech, :, :])
                wqk_sb.append((ech, w))
            wqk_sb = [w for _, w in sorted(wqk_sb)]
            csk_sb = const.tile([P, 2, N], FP16)
            nc.scalar.dma_start(csk_sb[:, :, :], csk_d.rearrange("p (a n) -> p a n", a=2))
            csq_sb = const.tile([P, 2, N], FP16)
            nc.scalar.dma_start(csq_sb[:, :, :], csq_d.rearrange("p (a n) -> p a n", a=2))
            ck_sb, sk_sb = csk_sb[:, 0, :], csk_sb[:, 1, :]
            cq_sb, sq_sb = csq_sb[:, 0, :], csq_sb[:, 1, :]
            wv_sb = const.tile([P, DC, 256], FP16)
            nc.scalar.dma_start(wv_sb[:, :, :], wv_d.rearrange("p (c e) -> p c e", c=DC))
            wo_sb = const.tile([P, NPAIR, DIM], FP16)
            nc.scalar.dma_start(wo_sb[:, :, :], wo_d.rearrange("p (a d) -> p a d", a=NPAIR))

            ones_f = const.tile([1, 64], F32)
            nc.vector.memset(ones_f[:, :], 1.0)
            ones_h = const.tile([1, 64], FP16)
            nc.vector.tensor_copy(ones_h[:, :], ones_f[:, :])
            nbias = const.tile([P, 1], F32)
            nc.vector.memset(nbias[:, :], -3.0)

            # ---------------- persistent tiles ----------------
            qrot = [[perst.tile([P, 512], FP16, name=f"qrot{p}_{t}")
                     for t in range(NT)] for p in range(NPAIR)]
            krot = [[perst.tile([P, 512], FP16, name=f"krot{p}_{t}")
                     for t in range(NT)] for p in range(NPAIR)]
            v_aug = [perst.tile([P, 4, HPC, 65], FP16, name=f"vaug{t}")
                     for t in range(NT)]
            for t in range(NT):
                nc.vector.memset(v_aug[t][:, :, :, 64:65], 1.0)
            aoT = [[perst.tile([P, 512], FP16, name=f"aoT{p}_{t}")
                    for t in range(NT)] for p in range(NPAIR)]

            # ---------------- helpers ----------------
            xT_r = xT_d.rearrange("p (t c n) -> p t c n", t=NT, c=DC)

            def load_x(t):
                # one contiguous 1MB DMA per n-tile (per-chunk DMAs pay ~1.7us
                # fixed cost each and serialize the sync ring)
                xt = tmp.tile([P, DC, 512], FP16, name="xt", tag="xt", bufs=2)
                nc.sync.dma_start(xt[:, :, :], xT_r[:, t, :, :])
                return xt

            def qk_chunk(ech, t, xt, dest, cos_sb, sin_sb):
                # qkvT e-chunk [128, 512] = W-chunk @ xT-tile, then rotary.
                pqk = ps.tile([P, 512], F32, name="pqk", tag="m", bufs=2)
                for c in range(DC):
                    nc.tensor.matmul(pqk[:, :],
                                     wqk_sb[ech][:, c, :],
                                     xt[:, c, :],
                                     start=(c == 0), stop=(c == DC - 1))
                sl = slice(t * 512, (t + 1) * 512)
                t1 = tmp.tile([P, 512], FP16, name="t1", tag="t1", bufs=2)
                t2 = tmp.tile([P, 512], FP16, name="t2", tag="t2", bufs=2)
                t3 = tmp.tile([P, 512], FP16, name="t3", tag="t3", bufs=2)
                nc.vector.tensor_tensor(t1[:, :], pqk[:, :], cos_sb[:, sl], op=MULT)
                nc.vector.tensor_tensor(t2[:, :], pqk[:, :], sin_sb[:, sl], op=MULT)
                nc.vector.stream_shuffle(t3[:, :], t2[:, :], PAIRSWAP)
                nc.vector.tensor_tensor(dest[:, :], t1[:, :], t3[:, :], op=ADD)

            def v_tile(t, xt):
                # v natural [n, e] for the 4 local heads, by 128-row subtiles
                for nsub in range(4):
                    pv = ps.tile([P, 256], F32, name="pv", tag="m", bufs=2)
                    for c in range(DC):
                        nc.tensor.matmul(pv[:, :],
                                         xt[:, c, nsub * P:(nsub + 1) * P],
                                         wv_sb[:, c, :],
                                         start=(c == 0), stop=(c == DC - 1))
                    nc.vector.tensor_copy(
                        v_aug[t][:, nsub, :, 0:64],
                        pv[:, :].rearrange("p (h d) -> p h d", h=HPC))

            def qkv_for_tile(t, ops):
                xt = load_x(t)
                for op in ops:
                    if op == "k0":
                        qk_chunk(2, t, xt, krot[0][t], ck_sb, sk_sb)
                    elif op == "k1":
                        qk_chunk(3, t, xt, krot[1][t], ck_sb, sk_sb)
                    elif op == "q0":
                        qk_chunk(0, t, xt, qrot[0][t], cq_sb, sq_sb)
                    elif op == "q1":
                        qk_chunk(1, t, xt, qrot[1][t], cq_sb, sq_sb)
                    elif op == "v":
                        v_tile(t, xt)

            def attention(nq, pair, pre_jb=None, mid_jb=None):
                pav = [ps.tile([65, 512], F32, name=f"pav{h}", tag="av", bufs=2)
                       for h in range(2)]
                for jb in range(JB):
                    if pre_jb is not None:
                        pre_jb(jb)
                    # per-jl tiles with the two heads adjacent: consecutive
                    # score matmuls then alternate PE row-groups (h0/h64) and
                    # write different psum banks, the layout concurrency needs
                    sc = [ps.tile([P, 2, 512], F32, name=f"sc{jl}", tag="s", bufs=2)
                          for jl in range(2)]
                    for jl in range(2):
                        jt = jb * 2 + jl
                        kt = krot[pair][jt // 4]
                        jsl = slice((jt % 4) * P, (jt % 4 + 1) * P)
                        for h in range(2):
                            rows = slice(h * 64, (h + 1) * 64)
                            nc.tensor.matmul(sc[jl][:, h, :],
                                             kt[rows, jsl],
                                             qrot[pair][nq][rows, :],
                                             start=True, stop=True,
                                             tile_position=(h * 64, 0))
                    ex = [tmp.tile([P, 2, 512], FP16, name=f"ex{jl}", tag="ex", bufs=4)
                          for jl in range(2)]
                    for jl in range(2):
                        # bias -3 keeps exp sums and the un-normalized AV psum
                        # inside fp16 range (max logit ~10.5); the denominator
                        # picks up the same factor so softmax is unchanged
                        nc.scalar.activation(ex[jl][:, :, :], sc[jl][:, :, :],
                                             EXP, bias=nbias[:, :])
                    if mid_jb is not None:
                        mid_jb(jb)
                    for jl in range(2):
                        jt = jb * 2 + jl
                        for h in range(2):
                            nc.tensor.matmul(pav[h][:, :],
                                             v_aug[jt // 4][:, jt % 4, pair * 2 + h, :],
                                             ex[jl][:, h, :],
                                             start=(jt == 0), stop=(jt == JTILES - 1))
                for h in range(2):
                    # evacuate psum immediately so the next (nq, pair) can start;
                    # sums row copied separately so it lands at partition 0
                    # (custom-DVE reciprocal_approx_fast requires base_partition 0)
                    av_sb = tmp.tile([64, 512], FP16, name="av_sb", tag="avs", bufs=3)
                    sm_sb = tmp.tile([1, 512], F32, name="sm_sb", tag="sms", bufs=4)
                    nc.vector.tensor_copy(av_sb[:, :], pav[h][0:64, :])
                    nc.vector.tensor_copy(sm_sb[:, :], pav[h][64:65, :])
                    rc = tmp.tile([1, 512], F32, name="rc", tag="rc", bufs=2)
                    nc.vector.reciprocal_approx_fast(rc[:, :], sm_sb[:, :])
                    rch = tmp.tile([1, 512], FP16, name="rch", tag="rch", bufs=2)
                    nc.vector.tensor_copy(rch[:, :], rc[:, :])
                    bc = tmp.tile([64, 512], FP16, name="bc", tag="bc", bufs=2)
                    if nq == NT - 1:
                        # tail-critical: broadcast via K=1 ones-matmul (no DMA
                        # round-trip latency before the last y projection)
                        pbc = ps.tile([64, 512], F32, name="pbc", tag="m", bufs=2)
                        nc.tensor.matmul(pbc[:, :], ones_h[:, :], rch[:, :],
                                         start=True, stop=True)
                        nc.vector.tensor_copy(bc[:, :], pbc[:, :])
                    else:
                        # broadcast across partitions via a DRAM round-trip
                        rd = dram.tile([1, 512], FP16, name="rd", tag="rd", bufs=2)
                        nc.sync.dma_start(rd[:, :], rch[:, :])
                        nc.sync.dma_start(bc[:, :], rd.to_broadcast([64, 512]))
                    rows = slice(h * 64, (h + 1) * 64)
                    nc.vector.tensor_tensor(aoT[pair][nq][rows, :],
                                            av_sb[:, :], bc[:, :], op=MULT)

            def y_proj_pair(nq, pair, out_d, row0):
                # single-pair partial projection (no cross-pair accumulation)
                for nsub in range(4):
                    ys = tmp.tile([P, DIM], FP16, name="ysp", tag="ys", bufs=2)
                    nsl = slice(nsub * P, (nsub + 1) * P)
                    for dh2 in range(2):
                        py = ps.tile([P, 512], F32, name="pyp", tag="m", bufs=2)
                        dsl = slice(dh2 * 512, (dh2 + 1) * 512)
                        nc.tensor.matmul(py[:, :], aoT[pair][nq][:, nsl],
                                         wo_sb[:, pair, dsl],
                                         start=True, stop=True)
                        nc.vector.tensor_copy(ys[:, dsl], py[:, :])
                    nc.scalar.dma_start(out_d[row0 + nsub * P:row0 + (nsub + 1) * P, :],
                                        ys[:, :])

            def y_proj(nq):
                for nsub in range(4):
                    ys = tmp.tile([P, DIM], FP16, name="ys", tag="ys", bufs=2)
                    nsl = slice(nsub * P, (nsub + 1) * P)
                    for dh2 in range(2):
                        py = ps.tile([P, 512], F32, name="py", tag="m", bufs=2)
                        dsl = slice(dh2 * 512, (dh2 + 1) * 512)
                        for pair in range(NPAIR):
                            nc.tensor.matmul(py[:, :],
                                             aoT[pair][nq][:, nsl],
                                             wo_sb[:, pair, dsl],
                                             start=(pair == 0), stop=(pair == NPAIR - 1))
                        nc.vector.tensor_copy(ys[:, dsl], py[:, :])
                    nc.scalar.dma_start(y_d[nq * 512 + nsub * P:
                                            nq * 512 + (nsub + 1) * P, :], ys[:, :])

            # ---------------- emission order ----------------
            # Tile has sequential program-order semantics: every tile must be
            # written (in emission order) before anything that reads it, and
            # per-psum-tag slot reuse is FIFO in emission order. QKV work and
            # the previous block's output projection are threaded just-in-time
            # through the attention j-loops: k before the dots that need it,
            # v between exp and the AV that needs it, next-q early (ahead of
            # y in the shared psum-tag FIFO) so rotary completes before the
            # block boundary.
            qkv_for_tile(0, ["k0", "q0"])

            def pre_first(jb):
                if jb == 1:
                    qkv_for_tile(0, ["k1", "q1"])
                elif jb in (2, 4, 6):
                    qkv_for_tile(jb // 2, ["k0", "k1"])

            def mid_first(jb):
                if jb in (0, 2, 4, 6):
                    qkv_for_tile(jb // 2, ["v"])

            def make_pre_q(t):
                def pre(jb):
                    if jb == 1:
                        qkv_for_tile(t, ["q0", "q1"])
                return pre

            def make_pre(nq):
                def pre(jb):
                    if jb == 1 and nq + 1 < NT:
                        qkv_for_tile(nq + 1, ["q0", "q1"])
                    if jb == 4 and nq >= 1:
                        y_proj(nq - 1)
                return pre

            for nq in range(NT):
                for pair in range(NPAIR):
                    if nq == 0 and pair == 0:
                        attention(nq, pair, pre_jb=pre_first, mid_jb=mid_first)
                    elif nq == 0 and pair == 1:
                        attention(nq, pair, pre_jb=make_pre_q(1))
                    elif pair == 0:
                        attention(nq, pair, pre_jb=make_pre(nq))
                    elif nq == NT - 1:
                        def pre_y3a(jb):
                            if jb == 2:
                                y_proj_pair(NT - 1, 0, y3a_d, 0)
                        attention(nq, pair, pre_jb=pre_y3a)
                    else:
                        attention(nq, pair)
            y_proj_pair(NT - 1, 1, y_d, (NT - 1) * 512)
    nc.compile()
    return nc


def _host_prep(x, rotary_emb, w_qkv, w_out):
    """Build the 8 per-core input maps (everything pre-laid-out + fp16)."""
    x = np.asarray(x, dtype=np.float32)
    rotary_emb = np.asarray(rotary_emb, dtype=np.float32)
    w_qkv = np.asarray(w_qkv, dtype=np.float32)
    w_out = np.asarray(w_out, dtype=np.float32)

    # interleaved dh permutation: new row 2i <- dim i, 2i+1 <- dim 32+i
    perm = np.empty(DH, dtype=np.int64)
    perm[0::2] = np.arange(32)
    perm[1::2] = np.arange(32) + 32
    pair_swap = np.arange(DH) ^ 1

    cos = np.cos(rotary_emb).T[perm]                      # [dh, n] permuted
    sin = np.sin(rotary_emb).T[perm]
    sign = np.where(perm < 32, -1.0, 1.0)[:, None].astype(np.float32)
    sin_eff = sign * sin
    sin_pre = sin_eff[pair_swap]                          # pre-swapped
    c2 = np.concatenate([cos, cos], axis=0)               # [128, n]
    s2 = np.concatenate([sin_pre, sin_pre], axis=0)
    csq = np.ascontiguousarray(np.concatenate(
        [SCALE * c2, SCALE * s2], axis=1).astype(np.float16))   # [128, 2*N]
    csk = np.ascontiguousarray(np.concatenate(
        [c2, s2], axis=1).astype(np.float16))

    in_maps = []
    for core in range(NCORES):
        b = core // (NCORES // B)
        g = core % (NCORES // B)
        heads = range(4 * g, 4 * g + HPC)
        q_rows = np.concatenate([h * DH + perm for h in heads])
        k_rows = np.concatenate([INNER + h * DH + perm for h in heads])
        v_rows = np.arange(2 * INNER + 4 * g * DH, 2 * INNER + (4 * g + HPC) * DH)

        # wqk: [p, ech, c, e] with ech 0/1 = q pairs, 2/3 = k pairs
        Wqk = w_qkv[np.concatenate([q_rows, k_rows])]     # [512, 1024]
        wqkh = Wqk.reshape(4, P, DC, P).transpose(3, 0, 2, 1)
        wqkh = np.ascontiguousarray(
            wqkh.reshape(P, 4 * DC * P).astype(np.float16))

        # wv: [p, c, e]
        Wv = w_qkv[v_rows]                                # [256, 1024]
        wvh = Wv.reshape(256, DC, P).transpose(2, 1, 0)
        wvh = np.ascontiguousarray(
            wvh.reshape(P, DC * 256).astype(np.float16))

        # wo: [p, pair, d]
        Wo = w_out[:, 4 * g * DH:(4 * g + HPC) * DH]      # [1024, 256]
        woh = Wo.T.reshape(NPAIR, P, DIM).transpose(1, 0, 2)
        woh = np.ascontiguousarray(
            woh.reshape(P, NPAIR * DIM).astype(np.float16))

        # x: [p, t, c, n']
        xh = x[b].reshape(NT, 512, DC, P).transpose(3, 0, 2, 1)
        xh = np.ascontiguousarray(
            xh.reshape(P, NT * DC * 512).astype(np.float16))

        in_maps.append({
            "xh": xh, "wqkh": wqkh, "wvh": wvh, "woh": woh,
            "csq": csq, "csk": csk,
        })
    return in_maps


def kernel(x, rotary_emb, w_qkv, w_out, b_out, _trace=False):
    if "nc" not in _CACHE:
        _CACHE["nc"] = _build()
    nc = _CACHE["nc"]
    in_maps = _host_prep(x, rotary_emb, w_qkv, w_out)
    res = run_bass_kernel_spmd(nc, in_maps, core_ids=list(range(NCORES)),
                               trace=_trace)
    _CACHE["last_result"] = res
    y = np.zeros((B, N, DIM), dtype=np.float32)
    for core in range(NCORES):
        b = core // (NCORES // B)
        y[b] += res.results[core]["y"].astype(np.float32)
        y[b, (NT - 1) * 512:] += res.results[core]["y3a"].astype(np.float32)
    y += np.asarray(b_out, dtype=np.float32)[None, None, :]
    return y
